# revision 4
# baseline (speedup 1.0000x reference)
"""NetVLAD pooling kernel for Trainium2, data-parallel over batch across 8 cores.

Computation per batch b (reference semantics):
  y      = x @ W_red.T + b_red            # [m, 64]
  yn     = y / ||y||_row                  # L2 normalize rows
  logits = yn @ W_lin.T + b_lin           # [m, 8]
  a      = softmax(logits, axis=1)
  vlad   = a.T @ yn - centroids * a.sum(0)[:, None]
  out    = l2norm_global(l2norm_rows(vlad).flatten())

Device-side algebra (per row m):
  yz   = x @ [W_red.T | W_red.T W_lin.T] + [b_red | W_lin b_red]   # fused [m, 72]
  inv  = exp(-0.5 ln(sum(y^2)))        # 1/||y|| via the ln/exp table set
  n    = exp(+0.5 ln(sum(y^2)))        # ||y||
  e    = exp(raw2 * inv)               # un-biased softmax numerator
  r    = 1 / sum_k(e * exp(b_lin))
  atil = e * (inv * r)                 # so atil.T @ [y | n] = [a.T yn | a.sum]
  vlad accumulated in PSUM; row k scaled by exp(b_lin)[k] at finalize.

x is shipped to the device pre-transposed to [b, C, m] bf16 so the contraction
dim (C) lands on SBUF partitions with contiguous DMA descriptors.
"""
import numpy as np
import ml_dtypes
from contextlib import ExitStack

import concourse.bass as bass
import concourse.tile as tile
import concourse.bass_isa as bass_isa
from concourse import bacc, mybir
from concourse._compat import with_exitstack
from concourse.bass_utils import run_bass_kernel_spmd

bf16 = ml_dtypes.bfloat16
F32 = mybir.dt.float32
BF16 = mybir.dt.bfloat16

N_CORES = 8
B, M, C = 32, 8192, 512
K, D = 8, 64
B_LOC = B // N_CORES          # 4 batches per core
M_TILE = 1024
N_TILES = M // M_TILE         # 8
SUB = M_TILE // 128           # 8 subtiles of 128 rows
NCH = C // 128                # 4 contraction chunks


@with_exitstack
def _netvlad_kernel(ctx: ExitStack, tc: tile.TileContext, out_d, xt_d, wcat_d,
                    bcat_d, eblbc_d, ebl8_d, cent_d):
    nc = tc.nc
    AF = mybir.ActivationFunctionType
    OP = mybir.AluOpType

    consts = ctx.enter_context(tc.tile_pool(name="consts", bufs=1))
    xt_pool = ctx.enter_context(tc.tile_pool(name="xt", bufs=3))
    sb = ctx.enter_context(tc.tile_pool(name="work", bufs=2))
    outp = ctx.enter_context(tc.tile_pool(name="outp", bufs=1))
    yz_pool = ctx.enter_context(tc.tile_pool(name="yz", bufs=2, space="PSUM"))
    vlad_pool = ctx.enter_context(tc.tile_pool(name="vlad", bufs=2, space="PSUM"))

    # constants, loaded once
    wcat = consts.tile([128, NCH, 72], BF16)
    nc.sync.dma_start(wcat[:], wcat_d.rearrange("j p t -> p j t"))
    bcat = consts.tile([1, 72], BF16)
    nc.sync.dma_start(bcat[:], bcat_d[:])
    eblbc = consts.tile([128, SUB, K], F32)
    nc.sync.dma_start(eblbc[:], eblbc_d[:])
    ebl8 = consts.tile([K, 1], F32)
    nc.sync.dma_start(ebl8[:], ebl8_d[:])
    cent = consts.tile([K, D], F32)
    nc.sync.dma_start(cent[:], cent_d[:])
    ones = consts.tile([1, 128], BF16)
    nc.vector.memset(ones[:], 1.0)

    outsb = outp.tile([K, B_LOC, D], F32)

    for b in range(B_LOC):
        vlad = vlad_pool.tile([K, D + 1], F32)
        xt_b = xt_d[b].rearrange("(j p) m -> p j m", p=128)
        for t in range(N_TILES):
            xt = xt_pool.tile([128, NCH, M_TILE], BF16)
            nc.sync.dma_start(xt[:], xt_b[:, :, t * M_TILE:(t + 1) * M_TILE])

            # fused reduction+logits matmul: yz[m, :72] = x @ Wcat + bcat
            yz = yz_pool.tile([128, SUB, 128], F32)
            for s in range(SUB):
                for j in range(NCH):
                    nc.tensor.matmul(
                        yz[:, s, :72],
                        xt[:, j, s * 128:(s + 1) * 128],
                        wcat[:, j, :],
                        start=(j == 0), stop=False,
                    )
                nc.tensor.matmul(yz[:, s, :72], ones[:], bcat[:],
                                 start=False, stop=True)

            # ss = sum(y^2) per row; inv = 1/||y||, n = ||y|| via ln/exp
            sqs = sb.tile([128, SUB, D], BF16)
            nc.scalar.activation(sqs[:], yz[:, :, :D], AF.Square)
            ss8 = sb.tile([128, SUB], F32)
            nc.vector.reduce_sum(ss8[:], sqs[:], axis=mybir.AxisListType.X)
            lss = sb.tile([128, SUB], F32)
            nc.scalar.activation(lss[:], ss8[:], AF.Ln)
            inv8 = sb.tile([128, SUB], F32)
            nc.scalar.activation(inv8[:], lss[:], AF.Exp, scale=-0.5)

            # agg rhs = [y | n] in bf16
            rhs = sb.tile([128, SUB, D + 1], BF16)
            nc.vector.tensor_copy(rhs[:, :, :D], yz[:, :, :D])
            nc.scalar.activation(rhs[:, :, D:D + 1], lss[:].unsqueeze(2),
                                 AF.Exp, scale=0.5)

            # softmax numerators: e = exp(raw2 * inv)
            t64 = sb.tile([128, SUB, K], F32)
            nc.vector.tensor_tensor(
                out=t64[:], in0=yz[:, :, D:D + K],
                in1=inv8[:].unsqueeze(2).broadcast_to([128, SUB, K]),
                op=OP.mult)
            e64 = sb.tile([128, SUB, K], F32)
            nc.scalar.activation(e64[:], t64[:], AF.Exp)
            # r = 1/sum_k(e * exp(b_lin)); q = inv * r
            am = sb.tile([128, SUB, K], F32)
            nc.vector.tensor_tensor(out=am[:], in0=e64[:], in1=eblbc[:], op=OP.mult)
            rs8 = sb.tile([128, SUB], F32)
            nc.vector.reduce_sum(rs8[:], am[:], axis=mybir.AxisListType.X)
            rr8 = sb.tile([128, SUB], F32)
            nc.vector.reciprocal(rr8[:], rs8[:])
            q8 = sb.tile([128, SUB], F32)
            nc.vector.tensor_tensor(out=q8[:], in0=inv8[:], in1=rr8[:], op=OP.mult)
            atil = sb.tile([128, SUB, K], BF16)
            nc.vector.tensor_tensor(
                out=atil[:], in0=e64[:],
                in1=q8[:].unsqueeze(2).broadcast_to([128, SUB, K]),
                op=OP.mult)

            # vlad[k, :] += atil_s.T @ [y | n]
            for s in range(SUB):
                nc.tensor.matmul(
                    vlad[:], atil[:, s, :], rhs[:, s, :],
                    start=(t == 0 and s == 0),
                    stop=(t == N_TILES - 1 and s == SUB - 1),
                )

        # finalize batch: vlad rows scaled by exp(b_lin), centroid subtract,
        # intra-normalize, global normalize
        vl = sb.tile([K, D + 1], F32)
        nc.vector.tensor_scalar_mul(vl[:], vlad[:], ebl8[:])
        cv = sb.tile([K, D], F32)
        nc.vector.tensor_scalar_mul(cv[:], cent[:], vl[:, D:D + 1])
        v = sb.tile([K, D], F32)
        nc.vector.tensor_sub(v[:], vl[:, :D], cv[:])
        sck = sb.tile([K, D], F32)
        nc.vector.tensor_tensor(out=sck[:], in0=v[:], in1=v[:], op=OP.mult)
        ssk = sb.tile([K, 1], F32)
        nc.vector.reduce_sum(ssk[:], sck[:], axis=mybir.AxisListType.X)
        lk = sb.tile([K, 1], F32)
        nc.scalar.activation(lk[:], ssk[:], AF.Ln)
        invk = sb.tile([K, 1], F32)
        nc.scalar.activation(invk[:], lk[:], AF.Exp, scale=-0.5)
        vn = sb.tile([K, D], F32)
        nc.vector.tensor_scalar_mul(vn[:], v[:], invk[:])
        sck2 = sb.tile([K, D], F32)
        nc.vector.tensor_tensor(out=sck2[:], in0=vn[:], in1=vn[:], op=OP.mult)
        gsp = sb.tile([K, 1], F32)
        nc.vector.reduce_sum(gsp[:], sck2[:], axis=mybir.AxisListType.X)
        gs = sb.tile([K, 1], F32)
        nc.gpsimd.partition_all_reduce(gs[:], gsp[:], channels=K,
                                       reduce_op=bass_isa.ReduceOp.add)
        lg = sb.tile([K, 1], F32)
        nc.scalar.activation(lg[:], gs[:], AF.Ln)
        ginv = sb.tile([K, 1], F32)
        nc.scalar.activation(ginv[:], lg[:], AF.Exp, scale=-0.5)
        nc.vector.tensor_scalar_mul(outsb[:, b, :], vn[:], ginv[:])

    nc.sync.dma_start(out_d.rearrange("b (k d) -> k b d", k=K), outsb[:])


_CACHE = {}


def _build_program():
    if "nc" in _CACHE:
        return _CACHE["nc"]
    nc = bacc.Bacc("TRN2", target_bir_lowering=False, debug=False,
                   num_devices=N_CORES)
    xt_d = nc.dram_tensor("xt", [B_LOC, C, M], BF16, kind="ExternalInput").ap()
    wcat_d = nc.dram_tensor("wcat", [NCH, 128, 72], BF16, kind="ExternalInput").ap()
    bcat_d = nc.dram_tensor("bcat", [1, 72], BF16, kind="ExternalInput").ap()
    eblbc_d = nc.dram_tensor("eblbc", [128, SUB, K], F32, kind="ExternalInput").ap()
    ebl8_d = nc.dram_tensor("ebl8", [K, 1], F32, kind="ExternalInput").ap()
    cent_d = nc.dram_tensor("cent", [K, D], F32, kind="ExternalInput").ap()
    out_d = nc.dram_tensor("out", [B_LOC, K * D], F32, kind="ExternalOutput").ap()

    with tile.TileContext(nc) as tc:
        _netvlad_kernel(tc, out_d, xt_d, wcat_d, bcat_d, eblbc_d, ebl8_d, cent_d)
    nc.compile()
    _CACHE["nc"] = nc
    return nc


def _prep_inputs(x, W_red, b_red, W_lin, b_lin, centroids):
    wcat = np.concatenate([W_red.T, W_red.T @ W_lin.T], axis=1)     # [512, 72]
    wcat = np.ascontiguousarray(wcat.astype(bf16).reshape(NCH, 128, 72))
    bcat = np.concatenate([b_red, W_lin @ b_red]).astype(bf16)[None, :]
    ebl = np.exp(b_lin).astype(np.float32)
    eblbc = np.ascontiguousarray(
        np.broadcast_to(ebl, (128, SUB, K)).astype(np.float32))
    ebl8 = ebl[:, None]
    cent = centroids.astype(np.float32)
    xt = np.ascontiguousarray(x.astype(bf16).transpose(0, 2, 1))    # [B, C, M]
    return xt, wcat, bcat, eblbc, ebl8, cent


def kernel(x, mask, W_red, b_red, W_lin, b_lin, centroids, **kwargs):
    x = np.asarray(x, dtype=np.float32)
    W_red = np.asarray(W_red, dtype=np.float32)
    b_red = np.asarray(b_red, dtype=np.float32)
    W_lin = np.asarray(W_lin, dtype=np.float32)
    b_lin = np.asarray(b_lin, dtype=np.float32)
    centroids = np.asarray(centroids, dtype=np.float32)

    xt, wcat, bcat, eblbc, ebl8, cent = _prep_inputs(
        x, W_red, b_red, W_lin, b_lin, centroids)

    nc = _build_program()
    in_maps = []
    for i in range(N_CORES):
        in_maps.append({
            "xt": np.ascontiguousarray(xt[i * B_LOC:(i + 1) * B_LOC]),
            "wcat": wcat, "bcat": bcat, "eblbc": eblbc,
            "ebl8": ebl8, "cent": cent,
        })
    res = run_bass_kernel_spmd(nc, in_maps, list(range(N_CORES)),
                               **kwargs.get("_run_kwargs", {}))
    out = np.concatenate([res.results[i]["out"] for i in range(N_CORES)], axis=0)
    if kwargs.get("_return_raw"):
        return out, res
    return out
